# revision 52
# baseline (speedup 1.0000x reference)
"""GQA attention kernel for Trainium2 (8 NeuronCores, Bass/Tile).

Problem: B=2, S=2048, D=3072, 24 Q heads / 8 KV heads, HD=128, RoPE,
additive causal mask, softmax, output projection.

Sharding: tensor-parallel over heads. Core h owns KV head h and Q heads
{3h, 3h+1, 3h+2} for BOTH batch elements. Each core produces a partial
y^T = wo_slice^T.T @ attn_out_heads^T of shape (B, D, S) in fp16; the
host sums the 8 partials in fp32 and transposes back.

Layout: everything stays transposed ([feature, token]) on chip so every
matmul contracts on the partition dim with a 512-wide fp16 moving
operand (1 cycle/row on the PE).

Key optimizations over the fp16 baseline:
  - attention probabilities e=exp(score*scale-3) and V are fp8e4 for
    fully-unmasked causal blocks, letting attn@V and the softmax row-sum
    matmuls use DoubleRow perf mode (two 128-row k-tiles per
    instruction, ~2x). Diagonal blocks stay fp16 so every row's softmax
    denominator is exact enough and can never flush to zero.
  - causal diagonal blocks are column-trimmed: scores/exp/attn@V only
    touch the valid triangle's column range; the triangle itself is
    masked with one shared 128x128 lower-triangular multiplier.
  - bulk DMA: one descriptor per x chunk / weight tensor / y chunk
    (~40 DMA issues total vs ~570), fp16 y partials.
  - RoPE entirely on the vector engine: rotate-half is expressed with
    cross-partition-base operand slices, no SBUF-SBUF DMAs and no
    scalar-engine copies; sin sign folded in on the host.
  - exp batched 2 causal blocks per activation instruction (reads a
    [128,1024] 2-bank PSUM window).
"""

import math
import os
import sys

import numpy as np

for _p in ("/opt/trn_rl_repo",):
    if os.path.isdir(_p) and _p not in sys.path:
        sys.path.insert(0, _p)

import ml_dtypes  # noqa: E402

import concourse.bass as bass  # noqa: E402
import concourse.mybir as mybir  # noqa: E402
import concourse.tile as tile  # noqa: E402
from concourse import bacc  # noqa: E402
from concourse.bass_utils import run_bass_kernel_spmd  # noqa: E402

F32 = mybir.dt.float32
F16 = mybir.dt.float16
F8 = mybir.dt.float8e4
AFT = mybir.ActivationFunctionType
DR = mybir.MatmulPerfMode.DoubleRow

N_CORES = 8

# Set by test harness to capture a profile on the next kernel() call.
TRACE = False
LAST_EXEC_NS = None
LAST_RESULTS = None

B, S, D = 2, 2048, 3072
QH, HD, SC = 3, 128, 512
CT = D // 128          # 24 contraction tiles for projections
KT = S // 128          # 16 key tiles
NSC = S // SC          # 4 token chunks
PB = SC // 128         # 4 key tiles per chunk
SCALE = 1.0 / math.sqrt(HD)
EXP_BIAS = -3.0        # uniform; cancels in softmax normalization

# fp8 prescales: wq/wk/wv/wo would be subnormal in e4m3 at their native
# 0.02 sigma, so weights carry x64 and activations x16 on chip. The x/w
# product scale (1024) is divided back out inside RoPE's cos/sin tables;
# V carries x16 via the transpose identity (I * 16/1024) so the
# attention output oh is x16, matching the fp8 wo path whose product
# scale (64*16 = 1024) the host divides out of the final reduction.
SX = 16.0              # x -> fp8
SW = 64.0              # wq/wk/wv/wo -> fp8 (wo16 also x64 to match)
SV = 16.0              # V (and hence oh) on-chip scale
# Q/K ride at SX*SW x true scale in fp16 (cos/sin stay full scale to
# dodge fp16 subnormals); exp's scale arg absorbs the dequant for free.
SCALE_EXP = SCALE / (SX * SW) ** 2
WO_FP8 = True          # heads 0,1 of wo in fp8 DoubleRow; head 2 fp16


def build_program():
    nc = bacc.Bacc("TRN2", target_bir_lowering=False, debug=False,
                   num_devices=N_CORES)

    # x is stored chunk-major [b, chunk, partition, ct, s] so each chunk
    # DMA reads 3KB+ contiguous DRAM per partition (512B segments from a
    # [B,D,S] layout ran at ~half DMA rate and stalled the QKV DR chain).
    xT = nc.declare_dram_parameter("xT", [B, NSC, 128, CT, SC], F8,
                                   isOutput=False)
    # tokens 0..SC-1 in fp16: chunk 0's projections run in fp16 so the
    # short causal rows (few softmax terms, no error averaging) stay
    # accurate; fp8 noise there would blow the early-row error up 5x.
    xT16 = nc.declare_dram_parameter("xT16", [B, 128, CT, SC], F16,
                                     isOutput=False)
    cosT = nc.declare_dram_parameter("cosT", [HD, S], F16, isOutput=False)
    sinT = nc.declare_dram_parameter("sinT", [HD, S], F16, isOutput=False)
    wq = nc.declare_dram_parameter("wq", [D, QH * HD], F8, isOutput=False)
    wk = nc.declare_dram_parameter("wk", [D, HD], F8, isOutput=False)
    wv = nc.declare_dram_parameter("wv", [D, HD], F8, isOutput=False)
    wq16 = nc.declare_dram_parameter("wq16", [D, QH * HD], F16,
                                     isOutput=False)
    wk16 = nc.declare_dram_parameter("wk16", [D, HD], F16, isOutput=False)
    wv16 = nc.declare_dram_parameter("wv16", [D, HD], F16, isOutput=False)
    # wo in both precisions: fp8 DoubleRow pair (heads 0,1) for the
    # diffuse-attention query chunks, full fp16 for qc0 whose
    # concentrated rows have large |oh| and can't absorb fp8 noise.
    wo8 = nc.declare_dram_parameter("wo8", [2 * HD, D], F8, isOutput=False)
    wo16 = nc.declare_dram_parameter("wo16", [QH * HD, D], F16,
                                     isOutput=False)
    tri = nc.declare_dram_parameter("tri", [128, 128], F16, isOutput=False)
    ident = nc.declare_dram_parameter("ident", [128, 128], F16, isOutput=False)
    ones8 = nc.declare_dram_parameter("ones8", [128, 2, 16], F8, isOutput=False)
    ones16 = nc.declare_dram_parameter("ones16", [128, 1], F16, isOutput=False)
    onesr = nc.declare_dram_parameter("onesr", [1, 128], F16, isOutput=False)
    ebias = nc.declare_dram_parameter("ebias", [128, 1], F32, isOutput=False)
    yT = nc.declare_dram_parameter("yT", [B, D, S], F16, isOutput=True)

    x_ap = xT.ap()
    x16_ap = xT16.ap()
    y_ap = yT.ap().rearrange("b (ct p) s -> b p ct s", p=128)

    with tile.TileContext(nc) as tc:
        from contextlib import ExitStack
        with ExitStack() as top:
            const = top.enter_context(tc.tile_pool(name="const", bufs=1))
            stream = top.enter_context(tc.tile_pool(name="stream", bufs=1))

            wq_sb = const.tile([128, CT, QH * HD], F8, name="wq_sb")
            wk_sb = const.tile([128, CT, HD], F8, name="wk_sb")
            wv_sb = const.tile([128, CT, HD], F8, name="wv_sb")
            wq16_sb = const.tile([128, CT, QH * HD], F16, name="wq16_sb")
            wk16_sb = const.tile([128, CT, HD], F16, name="wk16_sb")
            wv16_sb = const.tile([128, CT, HD], F16, name="wv16_sb")
            wo8_sb = const.tile([128, 2, D], F8, name="wo8_sb")
            wo16_sb = const.tile([128, QH, D], F16, name="wo16_sb")
            cos_sb = const.tile([128, S], F16, name="cos_sb")
            sin_sb = const.tile([128, S], F16, name="sin_sb")
            tri_sb = const.tile([128, 128], F16, name="tri_sb")
            ident_sb = const.tile([128, 128], F16, name="ident_sb")
            # [128, 2, 16] so the DoubleRow weights AP subtile step is
            # 16B-aligned (s3_lw_dual_fp8_restrictions); only col 0 is used.
            ones8_sb = const.tile([128, 2, 16], F8, name="ones8_sb")
            ones16_sb = const.tile([128, 1], F16, name="ones16_sb")
            onesr_sb = const.tile([1, 128], F16, name="onesr_sb")
            ebias_sb = const.tile([128, 1], F32, name="ebias_sb")

            # xt chunk prefetcher: one-chunk lookahead, 4 sub-DMAs per chunk
            # so the first matmuls can start before the whole chunk lands.
            xt_tiles = {}

            def ensure_xt(b, sc):
                if sc >= NSC or (b, sc) in xt_tiles:
                    return
                if sc == 0:
                    xt = stream.tile([128, CT, SC], F16, tag="xt16",
                                     bufs=1, name="xt16")
                    for c in range(0, CT, 6):
                        nc.sync.dma_start(xt[:, c:c + 6, :],
                                          x16_ap[b][:, c:c + 6, :])
                else:
                    xt = stream.tile([128, CT, SC], F8, tag="xt", bufs=2,
                                     name="xt")
                    for c in range(0, CT, 6):
                        nc.sync.dma_start(xt[:, c:c + 6, :],
                                          x_ap[b][sc][:, c:c + 6, :])
                xt_tiles[(b, sc)] = xt

            # DMA issue order: single sync HWDGE ring, sequenced so each
            # transfer lands just before its first consumer needs it.
            # The fp8 chunk-1 path starts first (small transfers) while the
            # 7MB of fp16 chunk-0 weights/x stream in behind it.
            nc.sync.dma_start(wk_sb[:], wk.ap().rearrange("(ct p) f -> p ct f", p=128))
            ensure_xt(0, 1)
            nc.sync.dma_start(wv_sb[:], wv.ap().rearrange("(ct p) f -> p ct f", p=128))
            nc.sync.dma_start(wq_sb[:], wq.ap().rearrange("(ct p) f -> p ct f", p=128))
            nc.sync.dma_start(cos_sb[:], cosT.ap())
            nc.sync.dma_start(sin_sb[:], sinT.ap())
            nc.sync.dma_start(wk16_sb[:],
                              wk16.ap().rearrange("(ct p) f -> p ct f", p=128))
            ensure_xt(0, 0)
            nc.sync.dma_start(wv16_sb[:],
                              wv16.ap().rearrange("(ct p) f -> p ct f", p=128))
            wq16_r = wq16.ap().rearrange("(ct p) f -> p ct f", p=128)
            for gi in (0, 1, 2):
                nc.sync.dma_start(wq16_sb[:, :, gi * HD:(gi + 1) * HD],
                                  wq16_r[:, :, gi * HD:(gi + 1) * HD])
            nc.sync.dma_start(tri_sb[:], tri.ap())
            nc.sync.dma_start(ident_sb[:], ident.ap())
            nc.sync.dma_start(ones8_sb[:], ones8.ap())
            nc.sync.dma_start(ones16_sb[:], ones16.ap())
            nc.sync.dma_start(onesr_sb[:], onesr.ap())
            nc.sync.dma_start(ebias_sb[:], ebias.ap())
            wo_loaded = [False]

            def load_wo():
                if not wo_loaded[0]:
                    wo_loaded[0] = True
                    nc.sync.dma_start(
                        wo8_sb[:],
                        wo8.ap().rearrange("(g p) d -> p g d", p=128))
                    nc.sync.dma_start(
                        wo16_sb[:],
                        wo16.ap().rearrange("(h p) d -> p h d", p=128))

            for b in range(B):
                with ExitStack() as bctx:
                    bpool = bctx.enter_context(
                        tc.tile_pool(name=f"b{b}_persist", bufs=1))
                    K_sb = bpool.tile([128, S], F16, name=f"K_sb{b}")
                    V16 = bpool.tile([128, KT, 128], F16, name=f"V16_{b}")
                    V8 = bpool.tile([128, KT, 128], F8, name=f"V8_{b}")
                    Q_sb = [bpool.tile([128, S], F16, name=f"Q_sb{b}_{h}")
                            for h in range(QH)]

                    # ---------------- QKV projection + RoPE ----------------
                    with ExitStack() as pctx:
                        pps = pctx.enter_context(
                            tc.tile_pool(name=f"b{b}_qkv_ps", bufs=1,
                                         space="PSUM"))
                        sp = pctx.enter_context(
                            tc.tile_pool(name=f"b{b}_qkv_sb", bufs=1))

                        def rope(acc, dst_slice, cs):
                            tmp1 = sp.tile([128, SC], F16, tag="t1", bufs=3,
                                           name="tmp1")
                            nc.vector.tensor_mul(tmp1[:], acc[:],
                                                 cos_sb[:, cs])
                            tmp2 = sp.tile([128, SC], F16, tag="t2", bufs=3,
                                           name="tmp2")
                            nc.vector.tensor_mul(tmp2[0:64, :],
                                                 acc[64:128, :],
                                                 sin_sb[0:64, cs])
                            nc.vector.tensor_mul(tmp2[64:128, :],
                                                 acc[0:64, :],
                                                 sin_sb[64:128, cs])
                            nc.vector.tensor_add(dst_slice, tmp1[:], tmp2[:])

                        # chunk 1 (fp8, small DMA) first so the PE starts
                        # ~1us in; chunk 0's 7MB of fp16 streams in behind.
                        SCO = (1, 0, 2, 3)
                        for si, sc in enumerate(SCO):
                            cs = slice(sc * SC, (sc + 1) * SC)
                            ensure_xt(b, sc)
                            xt = xt_tiles.pop((b, sc))
                            if si + 1 < NSC:
                                ensure_xt(b, SCO[si + 1])

                            # groups streamed one at a time: K, V, Q0..Q2
                            if sc == 0:
                                groups = [("k", wk16_sb, 0),
                                          ("v", wv16_sb, 0),
                                          ("q0", wq16_sb, 0),
                                          ("q1", wq16_sb, 1),
                                          ("q2", wq16_sb, 2)]
                            else:
                                groups = [("k", wk_sb, 0), ("v", wv_sb, 0),
                                          ("q0", wq_sb, 0),
                                          ("q1", wq_sb, 1),
                                          ("q2", wq_sb, 2)]
                            for gname, wsb, gi in groups:
                                acc = pps.tile([128, SC], F32, tag="acc",
                                               bufs=5, name="acc")
                                fs = slice(gi * HD, (gi + 1) * HD)
                                if sc == 0:
                                    for ct in range(CT):
                                        nc.tensor.matmul(
                                            acc[:], wsb[:, ct, fs],
                                            xt[:, ct, :],
                                            start=(ct == 0),
                                            stop=(ct == CT - 1))
                                else:
                                    for cp in range(CT // 2):
                                        nc.tensor.matmul(
                                            acc[:],
                                            wsb[:, 2 * cp:2 * cp + 2, fs],
                                            xt[:, 2 * cp:2 * cp + 2, :],
                                            start=(cp == 0),
                                            stop=(cp == CT // 2 - 1),
                                            perf_mode=DR)
                                if gname == "k":
                                    rope(acc, K_sb[:, cs], cs)
                                elif gname == "v":
                                    vstage = sp.tile([128, SC], F16,
                                                     tag="vst", bufs=2,
                                                     name="vstage")
                                    # dequant x/w prescales and apply the
                                    # on-chip V scale (transpose ignores the
                                    # identity's values, so scale here)
                                    nc.scalar.activation(
                                        vstage[:], acc[:], AFT.Copy,
                                        scale=SV / (SX * SW))
                                    vps = pps.tile([128, PB, 128], F16,
                                                   tag="vtr", bufs=2,
                                                   name="vps")
                                    for j in range(PB):
                                        nc.tensor.transpose(
                                            vps[:, j, :],
                                            vstage[:, j * 128:(j + 1) * 128],
                                            ident_sb[:])
                                    ks = slice(sc * PB, (sc + 1) * PB)
                                    nc.vector.tensor_copy(V16[:, ks, :],
                                                          vps[:])
                                    nc.vector.tensor_copy(V8[:, ks, :],
                                                          vps[:])
                                else:
                                    h = int(gname[1])
                                    rope(acc, Q_sb[h][:, cs], cs)

                    # ---------------- attention + out-projection ----------------
                    with ExitStack() as actx:
                        aps = actx.enter_context(
                            tc.tile_pool(name=f"b{b}_attn_ps", bufs=1,
                                         space="PSUM"))
                        asb = actx.enter_context(
                            tc.tile_pool(name=f"b{b}_attn_sb", bufs=1))

                        # wo first: it is needed ~25us into this attention
                        # phase; the next batch's x only at the next QKV.
                        load_wo()
                        if b + 1 < B:
                            ensure_xt(b + 1, 1)
                            ensure_xt(b + 1, 0)

                        def norm_pe(st):
                            """PE/ACT/DVE tail of softmax normalization.

                            Emitted *after* the next head's first score
                            matmuls so the broadcast matmul never stalls the
                            PE on the reciprocal chain."""
                            av, inv16, oh = st
                            invb = aps.tile([128, SC], F32, tag="av",
                                            bufs=3, name="invb")
                            nc.tensor.matmul(invb[:], onesr_sb[:],
                                             inv16[:], start=True, stop=True)
                            invb_sb = asb.tile([128, SC], F32, tag="invbsb",
                                               bufs=2, name="invb_sb")
                            nc.scalar.copy(invb_sb[:], invb[:])
                            nc.vector.tensor_mul(oh, av[:], invb_sb[:])

                        def emit_wo(qs, wo_dr, oh8p, oh16, oh_dsts):
                            # out-projection; y tiles rotate through the av
                            # psum tag so score-pair banks stay free for the
                            # next qc's attention. y stages through two
                            # 12-column half tiles to halve SBUF footprint.
                            y_all = None
                            for mt in range(CT):
                                if mt % 12 == 0:
                                    y_all = stream.tile([128, 12, SC], F16,
                                                        tag="yall", bufs=2,
                                                        name="y_half")
                                yp = aps.tile([128, SC], F32, tag="av",
                                              bufs=3, name="yp")
                                ms = slice(mt * 128, (mt + 1) * 128)
                                if wo_dr:
                                    nc.tensor.matmul(
                                        yp[:], wo8_sb[:, :, ms], oh8p[:],
                                        start=True, stop=False,
                                        perf_mode=DR, skip_group_check=True)
                                    nc.tensor.matmul(
                                        yp[:], wo16_sb[:, 2, ms], oh16[:],
                                        start=False, stop=True,
                                        skip_group_check=True)
                                else:
                                    for h in range(QH):
                                        nc.tensor.matmul(
                                            yp[:], wo16_sb[:, h, ms],
                                            oh_dsts[h],
                                            start=(h == 0),
                                            stop=(h == QH - 1))
                                col = mt % 12
                                if mt % 2 == 0:
                                    nc.vector.tensor_copy(y_all[:, col, :],
                                                          yp[:])
                                else:
                                    nc.scalar.copy(y_all[:, col, :], yp[:])
                                if mt in (5, 11, 17, 23):
                                    lo = {5: 0, 11: 6, 17: 12, 23: 18}[mt]
                                    nc.sync.dma_start(
                                        y_ap[b][:, lo:mt + 1, qs],
                                        y_all[:, lo % 12:col + 1, :])

                        # sparse qc0 between the dense qc2/qc3 phases; qc0's
                        # wo-loop is deferred past qc3's scores so its bare
                        # softmax latency (no full blocks to hide it) never
                        # stalls the PE.
                        deferred_wo = None
                        for qc in (1, 2, 0, 3):
                            qs = slice(qc * SC, (qc + 1) * SC)
                            wo_dr = WO_FP8 and qc != 0
                            if wo_dr:
                                oh8p = asb.tile([128, 2, SC], F8,
                                                tag="oh8", bufs=3,
                                                name="oh8p")
                                oh16 = asb.tile([128, SC], F16,
                                                tag="oh16", bufs=5,
                                                name="oh16")
                                oh_dsts = [oh8p[:, 0, :], oh8p[:, 1, :],
                                           oh16[:]]
                            else:
                                oh_dsts = [asb.tile([128, SC], F16,
                                                    tag="oh16", bufs=5,
                                                    name="oh")[:]
                                           for _ in range(QH)]
                            pending = None
                            for h in range(QH):
                                av = aps.tile([128, SC], F32, tag="av",
                                              bufs=3, name="av")
                                r = aps.tile([1, SC], F32, tag="r", bufs=1,
                                             name="r")
                                # full (unmasked) kt pairs: kts 0..4qc-1, fp8 DR
                                for p in range(2 * qc):
                                    s2 = aps.tile([128, 2 * SC], F32,
                                                  tag="sp", bufs=2, name="s2")
                                    for half in (0, 1):
                                        kt = 2 * p + half
                                        nc.tensor.matmul(
                                            s2[:, half * SC:(half + 1) * SC],
                                            K_sb[:, kt * 128:(kt + 1) * 128],
                                            Q_sb[h][:, qs],
                                            start=True, stop=True)
                                    if pending is not None:
                                        norm_pe(pending)
                                        pending = None
                                    e2 = asb.tile([128, 2, SC], F8, tag="e8",
                                                  bufs=4, name="e2")
                                    nc.scalar.activation(
                                        e2[:].rearrange("p a s -> p (a s)"),
                                        s2[:], AFT.Exp,
                                        scale=SCALE_EXP, bias=ebias_sb[:])
                                    st = (p == 0)
                                    nc.tensor.matmul(
                                        av[:], V8[:, 2 * p:2 * p + 2, :],
                                        e2[:], start=st, stop=False,
                                        perf_mode=DR, skip_group_check=True)
                                    nc.tensor.matmul(
                                        r[:], ones8_sb[:, :, 0:1], e2[:],
                                        start=st, stop=False,
                                        perf_mode=DR, skip_group_check=True)

                                # diagonal kts 4qc..4qc+3: fp16, col-trimmed,
                                # two kts packed contiguously per psum pair
                                # tile so one exp covers both.
                                sD = eD2 = None
                                for i in range(4):
                                    kt = 4 * qc + i
                                    thr = 128 * i
                                    w = SC - thr
                                    if i % 2 == 0:
                                        sD = aps.tile([128, 2 * SC], F32,
                                                      tag="sp", bufs=2,
                                                      name="sD")
                                        eD2 = asb.tile([128, 2 * SC], F16,
                                                       tag="e16", bufs=4,
                                                       name="eD2")
                                        base = 0
                                    else:
                                        base = SC - 128 * (i - 1)  # w_even
                                    nc.tensor.matmul(
                                        sD[:, base:base + w],
                                        K_sb[:, kt * 128:(kt + 1) * 128],
                                        Q_sb[h][:, qc * SC + thr:
                                                (qc + 1) * SC],
                                        start=True, stop=True)
                                    if pending is not None:
                                        norm_pe(pending)
                                        pending = None
                                    if i % 2 == 1:
                                        # one exp over both packed regions
                                        nc.scalar.activation(
                                            eD2[:, 0:base + w],
                                            sD[:, 0:base + w], AFT.Exp,
                                            scale=SCALE_EXP, bias=ebias_sb[:])
                                    for ii in (i - 1, i) if i % 2 else ():
                                        tt = 128 * ii
                                        bb = 0 if ii % 2 == 0 else base
                                        ww = SC - tt
                                        nc.vector.tensor_mul(
                                            eD2[:, bb:bb + 128],
                                            eD2[:, bb:bb + 128], tri_sb[:])
                                        st = (qc == 0 and ii == 0)
                                        sp_ = (ii == 3)
                                        nc.tensor.matmul(
                                            av[:, tt:SC],
                                            V16[:, 4 * qc + ii, :],
                                            eD2[:, bb:bb + ww],
                                            start=st, stop=sp_,
                                            skip_group_check=True)
                                        nc.tensor.matmul(
                                            r[:, tt:SC], ones16_sb[:],
                                            eD2[:, bb:bb + ww],
                                            start=st, stop=sp_,
                                            skip_group_check=True)

                                # softmax normalization: DVE part now, PE
                                # part deferred into the next head
                                inv = asb.tile([1, SC], F32, tag="inv",
                                               bufs=2, name="inv")
                                nc.vector.reciprocal_approx_fast(inv[:], r[:])
                                inv16 = asb.tile([1, SC], F16, tag="inv16",
                                                 bufs=2, name="inv16")
                                nc.vector.tensor_copy(inv16[:], inv[:])
                                pending = (av, inv16, oh_dsts[h])
                            norm_pe(pending)

                            wo_args = (qs, wo_dr,
                                       oh8p if wo_dr else None,
                                       oh16 if wo_dr else None, oh_dsts)
                            if qc == 0:
                                deferred_wo = wo_args
                            else:
                                if deferred_wo is not None:
                                    emit_wo(*deferred_wo)
                                    deferred_wo = None
                                emit_wo(*wo_args)

    nc.compile()
    return nc


def make_inputs(x, freqs_cos, freqs_sin, mask, wq, wk, wv, wo):
    """Host-side preprocessing -> per-core input maps."""
    f32, f16 = np.float32, np.float16
    f8 = ml_dtypes.float8_e4m3

    x = np.asarray(x, f32)
    xs = (x * SX).reshape(B, NSC, SC, CT, 128)
    # chunk-major [b, chunk, partition, ct, s]: 3KB+ contiguous DRAM per
    # partition per chunk sub-DMA
    xT = np.ascontiguousarray(np.transpose(xs, (0, 1, 4, 3, 2)).astype(f8))
    xT16 = np.ascontiguousarray(
        np.transpose(xs[:, 0], (0, 3, 2, 1)).astype(f16))
    cosT = np.ascontiguousarray(
        np.concatenate([freqs_cos, freqs_cos], axis=1).T.astype(f16))
    sinT = np.concatenate([freqs_sin, freqs_sin], axis=1).T.astype(f32).copy()
    sinT[:HD // 2] *= -1.0  # sign of rotate-half folded in
    sinT = np.ascontiguousarray(sinT.astype(f16))

    # sanity: mask must be the causal tril mask the kernel hardcodes
    m = np.asarray(mask, f32)[0, 0]
    assert (m[np.tril_indices(4)] == 0).all() and m[0, 1] < -1e8, "non-causal mask"

    tri = np.ascontiguousarray(
        (np.arange(128)[None, :] >= np.arange(128)[:, None]).astype(f16))
    identity = np.ascontiguousarray(np.eye(128, dtype=f16))

    wqTs = np.asarray(wq, f32).T * SW
    wkTs = np.asarray(wk, f32).T * SW
    wvTs = np.asarray(wv, f32).T * SW
    wqT, wqT16 = wqTs.astype(f8), wqTs.astype(f16)
    wkT, wkT16 = wkTs.astype(f8), wkTs.astype(f16)
    wvT, wvT16 = wvTs.astype(f8), wvTs.astype(f16)
    woT = np.asarray(wo, f32).T * SW

    in_maps = []
    for h in range(N_CORES):
        qsl = slice(h * QH * HD, (h + 1) * QH * HD)
        ksl = slice(h * HD, (h + 1) * HD)
        im = {
            "xT": xT,
            "xT16": xT16,
            "cosT": cosT,
            "sinT": sinT,
            "wq": np.ascontiguousarray(wqT[:, qsl]),
            "wk": np.ascontiguousarray(wkT[:, ksl]),
            "wv": np.ascontiguousarray(wvT[:, ksl]),
            "wq16": np.ascontiguousarray(wqT16[:, qsl]),
            "wk16": np.ascontiguousarray(wkT16[:, ksl]),
            "wv16": np.ascontiguousarray(wvT16[:, ksl]),
            "tri": tri,
            "ident": identity,
            "ones8": np.ones((128, 2, 16), f8),
            "ones16": np.ones((128, 1), f16),
            "onesr": np.ones((1, 128), f16),
            "ebias": np.full((128, 1), EXP_BIAS, f32),
        }
        wo_core = woT[qsl, :]
        im["wo8"] = np.ascontiguousarray(wo_core[:2 * HD, :]).astype(f8)
        im["wo16"] = np.ascontiguousarray(wo_core).astype(f16)
        in_maps.append(im)
    return in_maps


_CACHE = {}


def kernel(x, freqs_cos, freqs_sin, mask, wq, wk, wv, wo):
    global LAST_EXEC_NS, LAST_RESULTS
    assert tuple(x.shape) == (B, S, D), x.shape

    in_maps = make_inputs(x, freqs_cos, freqs_sin, mask, wq, wk, wv, wo)

    if "nc" not in _CACHE:
        _CACHE["nc"] = build_program()
    nc = _CACHE["nc"]

    kwargs = {}
    if TRACE:
        kwargs = dict(trace=True, trace_cores=[0])
    res = run_bass_kernel_spmd(nc, in_maps, list(range(N_CORES)), **kwargs)
    LAST_EXEC_NS = res.exec_time_ns
    LAST_RESULTS = res

    acc = np.zeros((B, D, S), np.float32)
    for i in range(N_CORES):
        acc += res.results[i]["yT"].astype(np.float32)
    acc *= 1.0 / (SW * SV)
    y = np.ascontiguousarray(np.transpose(acc, (0, 2, 1)))
    return y



# revision 56
# speedup vs baseline: 1.1834x; 1.1834x over previous
"""GQA attention kernel for Trainium2 (8 NeuronCores, Bass/Tile).

Problem: B=2, S=2048, D=3072, 24 Q heads / 8 KV heads, HD=128, RoPE,
additive causal mask, softmax, output projection.

Sharding: tensor-parallel over heads. Core h owns KV head h and Q heads
{3h, 3h+1, 3h+2} for BOTH batch elements. Each core produces a partial
y^T = wo_slice^T.T @ attn_out_heads^T of shape (B, D, S) in fp16; the
host sums the 8 partials in fp32 and transposes back.

Layout: everything stays transposed ([feature, token]) on chip so every
matmul contracts on the partition dim with a 512-wide fp16 moving
operand (1 cycle/row on the PE).

Key optimizations over the fp16 baseline:
  - attention probabilities e=exp(score*scale-3) and V are fp8e4 for
    fully-unmasked causal blocks, letting attn@V and the softmax row-sum
    matmuls use DoubleRow perf mode (two 128-row k-tiles per
    instruction, ~2x). Diagonal blocks stay fp16 so every row's softmax
    denominator is exact enough and can never flush to zero.
  - causal diagonal blocks are column-trimmed: scores/exp/attn@V only
    touch the valid triangle's column range; the triangle itself is
    masked with one shared 128x128 lower-triangular multiplier.
  - bulk DMA: one descriptor per x chunk / weight tensor / y chunk
    (~40 DMA issues total vs ~570), fp16 y partials.
  - RoPE entirely on the vector engine: rotate-half is expressed with
    cross-partition-base operand slices, no SBUF-SBUF DMAs and no
    scalar-engine copies; sin sign folded in on the host.
  - exp batched 2 causal blocks per activation instruction (reads a
    [128,1024] 2-bank PSUM window).
"""

import math
import os
import sys

import numpy as np

for _p in ("/opt/trn_rl_repo",):
    if os.path.isdir(_p) and _p not in sys.path:
        sys.path.insert(0, _p)

import ml_dtypes  # noqa: E402

import concourse.bass as bass  # noqa: E402
import concourse.mybir as mybir  # noqa: E402
import concourse.tile as tile  # noqa: E402
from concourse import bacc  # noqa: E402
from concourse.bass_utils import run_bass_kernel_spmd  # noqa: E402

F32 = mybir.dt.float32
F16 = mybir.dt.float16
F8 = mybir.dt.float8e4
AFT = mybir.ActivationFunctionType
DR = mybir.MatmulPerfMode.DoubleRow

N_CORES = 8

# Set by test harness to capture a profile on the next kernel() call.
TRACE = False
LAST_EXEC_NS = None
LAST_RESULTS = None

B, S, D = 2, 2048, 3072
QH, HD, SC = 3, 128, 512
CT = D // 128          # 24 contraction tiles for projections
KT = S // 128          # 16 key tiles
NSC = S // SC          # 4 token chunks
PB = SC // 128         # 4 key tiles per chunk
SCALE = 1.0 / math.sqrt(HD)
EXP_BIAS = -3.0        # uniform; cancels in softmax normalization

# fp8 prescales: wq/wk/wv/wo would be subnormal in e4m3 at their native
# 0.02 sigma, so weights carry x64 and activations x16 on chip. The x/w
# product scale (1024) is divided back out inside RoPE's cos/sin tables;
# V carries x16 via the transpose identity (I * 16/1024) so the
# attention output oh is x16, matching the fp8 wo path whose product
# scale (64*16 = 1024) the host divides out of the final reduction.
SX = 16.0              # x -> fp8
SW = 64.0              # wq/wk/wv/wo -> fp8 (wo16 also x64 to match)
SV = 16.0              # V (and hence oh) on-chip scale
# Q/K ride at SX*SW x true scale in fp16 (cos/sin stay full scale to
# dodge fp16 subnormals); exp's scale arg absorbs the dequant for free.
SCALE_EXP = SCALE / (SX * SW) ** 2
WO_FP8 = True          # heads 0,1 of wo in fp8 DoubleRow; head 2 fp16


def build_program():
    nc = bacc.Bacc("TRN2", target_bir_lowering=False, debug=False,
                   num_devices=N_CORES)

    # x is stored chunk-major [b, chunk, partition, ct, s] so each chunk
    # DMA reads 3KB+ contiguous DRAM per partition (512B segments from a
    # [B,D,S] layout ran at ~half DMA rate and stalled the QKV DR chain).
    xT = nc.declare_dram_parameter("xT", [B, NSC, 128, CT, SC], F8,
                                   isOutput=False)
    # tokens 0..SC-1 in fp16: chunk 0's projections run in fp16 so the
    # short causal rows (few softmax terms, no error averaging) stay
    # accurate; fp8 noise there would blow the early-row error up 5x.
    xT16 = nc.declare_dram_parameter("xT16", [B, 128, CT, SC], F16,
                                     isOutput=False)
    cosT = nc.declare_dram_parameter("cosT", [HD, S], F16, isOutput=False)
    sinT = nc.declare_dram_parameter("sinT", [HD, S], F16, isOutput=False)
    wq = nc.declare_dram_parameter("wq", [D, QH * HD], F8, isOutput=False)
    wk = nc.declare_dram_parameter("wk", [D, HD], F8, isOutput=False)
    wv = nc.declare_dram_parameter("wv", [D, HD], F8, isOutput=False)
    wq16 = nc.declare_dram_parameter("wq16", [D, QH * HD], F16,
                                     isOutput=False)
    wk16 = nc.declare_dram_parameter("wk16", [D, HD], F16, isOutput=False)
    wv16 = nc.declare_dram_parameter("wv16", [D, HD], F16, isOutput=False)
    # wo in both precisions: fp8 DoubleRow pair (heads 0,1) for the
    # diffuse-attention query chunks, full fp16 for qc0 whose
    # concentrated rows have large |oh| and can't absorb fp8 noise.
    wo8 = nc.declare_dram_parameter("wo8", [2 * HD, D], F8, isOutput=False)
    wo16 = nc.declare_dram_parameter("wo16", [QH * HD, D], F16,
                                     isOutput=False)
    tri = nc.declare_dram_parameter("tri", [128, 128], F16, isOutput=False)
    ident = nc.declare_dram_parameter("ident", [128, 128], F16, isOutput=False)
    ones8 = nc.declare_dram_parameter("ones8", [128, 2, 16], F8, isOutput=False)
    ones16 = nc.declare_dram_parameter("ones16", [128, 1], F16, isOutput=False)
    onesr = nc.declare_dram_parameter("onesr", [1, 128], F16, isOutput=False)
    ebias = nc.declare_dram_parameter("ebias", [128, 1], F32, isOutput=False)
    yT = nc.declare_dram_parameter("yT", [B, D, S], F16, isOutput=True)

    x_ap = xT.ap()
    x16_ap = xT16.ap()
    y_ap = yT.ap().rearrange("b (ct p) s -> b p ct s", p=128)

    with tile.TileContext(nc) as tc:
        from contextlib import ExitStack
        with ExitStack() as top:
            const = top.enter_context(tc.tile_pool(name="const", bufs=1))
            stream = top.enter_context(tc.tile_pool(name="stream", bufs=1))

            wq_sb = const.tile([128, CT, QH * HD], F8, name="wq_sb")
            wk_sb = const.tile([128, CT, HD], F8, name="wk_sb")
            wv_sb = const.tile([128, CT, HD], F8, name="wv_sb")
            wq16_sb = const.tile([128, CT, QH * HD], F16, name="wq16_sb")
            wk16_sb = const.tile([128, CT, HD], F16, name="wk16_sb")
            wv16_sb = const.tile([128, CT, HD], F16, name="wv16_sb")
            wo8_sb = const.tile([128, 2, D], F8, name="wo8_sb")
            wo16_sb = const.tile([128, QH, D], F16, name="wo16_sb")
            cos_sb = const.tile([128, S], F16, name="cos_sb")
            sin_sb = const.tile([128, S], F16, name="sin_sb")
            tri_sb = const.tile([128, 128], F16, name="tri_sb")
            ident_sb = const.tile([128, 128], F16, name="ident_sb")
            # [128, 2, 16] so the DoubleRow weights AP subtile step is
            # 16B-aligned (s3_lw_dual_fp8_restrictions); only col 0 is used.
            ones8_sb = const.tile([128, 2, 16], F8, name="ones8_sb")
            ones16_sb = const.tile([128, 1], F16, name="ones16_sb")
            onesr_sb = const.tile([1, 128], F16, name="onesr_sb")
            ebias_sb = const.tile([128, 1], F32, name="ebias_sb")

            # xt chunk prefetcher: one-chunk lookahead, 4 sub-DMAs per chunk
            # so the first matmuls can start before the whole chunk lands.
            xt_tiles = {}

            def ensure_xt(b, sc):
                if sc >= NSC or (b, sc) in xt_tiles:
                    return
                if sc == 0:
                    xt = stream.tile([128, CT, SC], F16, tag="xt16",
                                     bufs=1, name="xt16")
                    for c in range(0, CT, 6):
                        nc.sync.dma_start(xt[:, c:c + 6, :],
                                          x16_ap[b][:, c:c + 6, :])
                else:
                    xt = stream.tile([128, CT, SC], F8, tag="xt", bufs=2,
                                     name="xt")
                    for c in range(0, CT, 6):
                        nc.sync.dma_start(xt[:, c:c + 6, :],
                                          x_ap[b][sc][:, c:c + 6, :])
                xt_tiles[(b, sc)] = xt

            # DMA issue order: single sync HWDGE ring, sequenced so each
            # transfer lands just before its first consumer needs it.
            # The fp8 chunk-1 path starts first (small transfers) while the
            # 7MB of fp16 chunk-0 weights/x stream in behind it.
            nc.sync.dma_start(wk_sb[:], wk.ap().rearrange("(ct p) f -> p ct f", p=128))
            ensure_xt(0, 1)
            nc.sync.dma_start(wv_sb[:], wv.ap().rearrange("(ct p) f -> p ct f", p=128))
            nc.sync.dma_start(wq_sb[:], wq.ap().rearrange("(ct p) f -> p ct f", p=128))
            nc.sync.dma_start(cos_sb[:], cosT.ap())
            nc.sync.dma_start(sin_sb[:], sinT.ap())
            nc.sync.dma_start(wk16_sb[:],
                              wk16.ap().rearrange("(ct p) f -> p ct f", p=128))
            ensure_xt(0, 0)
            nc.sync.dma_start(wv16_sb[:],
                              wv16.ap().rearrange("(ct p) f -> p ct f", p=128))
            wq16_r = wq16.ap().rearrange("(ct p) f -> p ct f", p=128)
            for gi in (0, 1, 2):
                nc.sync.dma_start(wq16_sb[:, :, gi * HD:(gi + 1) * HD],
                                  wq16_r[:, :, gi * HD:(gi + 1) * HD])
            nc.sync.dma_start(tri_sb[:], tri.ap())
            nc.sync.dma_start(ident_sb[:], ident.ap())
            nc.sync.dma_start(ones8_sb[:], ones8.ap())
            nc.sync.dma_start(ones16_sb[:], ones16.ap())
            nc.sync.dma_start(onesr_sb[:], onesr.ap())
            nc.sync.dma_start(ebias_sb[:], ebias.ap())
            wo_loaded = [False]

            def load_wo():
                if not wo_loaded[0]:
                    wo_loaded[0] = True
                    nc.sync.dma_start(
                        wo8_sb[:],
                        wo8.ap().rearrange("(g p) d -> p g d", p=128))
                    nc.sync.dma_start(
                        wo16_sb[:],
                        wo16.ap().rearrange("(h p) d -> p h d", p=128))

            for b in range(B):
                with ExitStack() as bctx:
                    bpool = bctx.enter_context(
                        tc.tile_pool(name=f"b{b}_persist", bufs=1))
                    K_sb = bpool.tile([128, S], F16, name=f"K_sb{b}")
                    V16 = bpool.tile([128, KT, 128], F16, name=f"V16_{b}")
                    V8 = bpool.tile([128, KT, 128], F8, name=f"V8_{b}")
                    Q_sb = [bpool.tile([128, S], F16, name=f"Q_sb{b}_{h}")
                            for h in range(QH)]

                    # ---------------- QKV projection + RoPE ----------------
                    with ExitStack() as pctx:
                        pps = pctx.enter_context(
                            tc.tile_pool(name=f"b{b}_qkv_ps", bufs=1,
                                         space="PSUM"))
                        sp = pctx.enter_context(
                            tc.tile_pool(name=f"b{b}_qkv_sb", bufs=1))

                        def rope(acc, dst_slice, cs):
                            tmp1 = sp.tile([128, SC], F16, tag="t1", bufs=3,
                                           name="tmp1")
                            nc.vector.tensor_mul(tmp1[:], acc[:],
                                                 cos_sb[:, cs])
                            tmp2 = sp.tile([128, SC], F16, tag="t2", bufs=3,
                                           name="tmp2")
                            nc.vector.tensor_mul(tmp2[0:64, :],
                                                 acc[64:128, :],
                                                 sin_sb[0:64, cs])
                            nc.vector.tensor_mul(tmp2[64:128, :],
                                                 acc[0:64, :],
                                                 sin_sb[64:128, cs])
                            nc.vector.tensor_add(dst_slice, tmp1[:], tmp2[:])

                        # chunk 1 (fp8, small DMA) first so the PE starts
                        # ~1us in; chunk 0's 7MB of fp16 streams in behind.
                        SCO = (1, 0, 2, 3)
                        for si, sc in enumerate(SCO):
                            cs = slice(sc * SC, (sc + 1) * SC)
                            ensure_xt(b, sc)
                            xt = xt_tiles.pop((b, sc))
                            if si + 1 < NSC:
                                ensure_xt(b, SCO[si + 1])

                            # groups streamed one at a time: K, V, Q0..Q2
                            if sc == 0:
                                groups = [("k", wk16_sb, 0),
                                          ("v", wv16_sb, 0),
                                          ("q0", wq16_sb, 0),
                                          ("q1", wq16_sb, 1),
                                          ("q2", wq16_sb, 2)]
                            else:
                                groups = [("k", wk_sb, 0), ("v", wv_sb, 0),
                                          ("q0", wq_sb, 0),
                                          ("q1", wq_sb, 1),
                                          ("q2", wq_sb, 2)]
                            for gname, wsb, gi in groups:
                                acc = pps.tile([128, SC], F32, tag="acc",
                                               bufs=5, name="acc")
                                fs = slice(gi * HD, (gi + 1) * HD)
                                if sc == 0:
                                    for ct in range(CT):
                                        nc.tensor.matmul(
                                            acc[:], wsb[:, ct, fs],
                                            xt[:, ct, :],
                                            start=(ct == 0),
                                            stop=(ct == CT - 1))
                                else:
                                    for cp in range(CT // 2):
                                        nc.tensor.matmul(
                                            acc[:],
                                            wsb[:, 2 * cp:2 * cp + 2, fs],
                                            xt[:, 2 * cp:2 * cp + 2, :],
                                            start=(cp == 0),
                                            stop=(cp == CT // 2 - 1),
                                            perf_mode=DR)
                                if gname == "k":
                                    rope(acc, K_sb[:, cs], cs)
                                elif gname == "v":
                                    vstage = sp.tile([128, SC], F16,
                                                     tag="vst", bufs=2,
                                                     name="vstage")
                                    # dequant x/w prescales and apply the
                                    # on-chip V scale (transpose ignores the
                                    # identity's values, so scale here)
                                    nc.scalar.activation(
                                        vstage[:], acc[:], AFT.Copy,
                                        scale=SV / (SX * SW))
                                    vps = pps.tile([128, PB, 128], F16,
                                                   tag="vtr", bufs=2,
                                                   name="vps")
                                    for j in range(PB):
                                        nc.tensor.transpose(
                                            vps[:, j, :],
                                            vstage[:, j * 128:(j + 1) * 128],
                                            ident_sb[:])
                                    ks = slice(sc * PB, (sc + 1) * PB)
                                    nc.vector.tensor_copy(V16[:, ks, :],
                                                          vps[:])
                                    nc.vector.tensor_copy(V8[:, ks, :],
                                                          vps[:])
                                else:
                                    h = int(gname[1])
                                    rope(acc, Q_sb[h][:, cs], cs)

                    # ---------------- attention + out-projection ----------------
                    with ExitStack() as actx:
                        aps = actx.enter_context(
                            tc.tile_pool(name=f"b{b}_attn_ps", bufs=1,
                                         space="PSUM"))
                        asb = actx.enter_context(
                            tc.tile_pool(name=f"b{b}_attn_sb", bufs=1))

                        # wo first: it is needed ~25us into this attention
                        # phase; the next batch's x only at the next QKV.
                        load_wo()
                        if b + 1 < B:
                            ensure_xt(b + 1, 1)
                            ensure_xt(b + 1, 0)

                        def norm_pe(st):
                            """PE/ACT/DVE tail of softmax normalization.

                            Emitted *after* the next head's first score
                            matmuls so the broadcast matmul never stalls the
                            PE on the reciprocal chain."""
                            av, inv16, oh = st
                            invb = aps.tile([128, SC], F32, tag="av",
                                            bufs=3, name="invb")
                            nc.tensor.matmul(invb[:], onesr_sb[:],
                                             inv16[:], start=True, stop=True)
                            invb_sb = asb.tile([128, SC], F32, tag="invbsb",
                                               bufs=2, name="invb_sb")
                            nc.scalar.copy(invb_sb[:], invb[:])
                            nc.vector.tensor_mul(oh, av[:], invb_sb[:])

                        def emit_wo(qs, wo_dr, oh8p, oh16, oh_dsts):
                            # out-projection; y tiles rotate through the av
                            # psum tag so score-pair banks stay free for the
                            # next qc's attention. y stages through two
                            # 12-column half tiles to halve SBUF footprint.
                            y_all = None
                            for mt in range(CT):
                                if mt % 12 == 0:
                                    y_all = stream.tile([128, 12, SC], F16,
                                                        tag="yall", bufs=2,
                                                        name="y_half")
                                yp = aps.tile([128, SC], F32, tag="av",
                                              bufs=3, name="yp")
                                ms = slice(mt * 128, (mt + 1) * 128)
                                if wo_dr:
                                    nc.tensor.matmul(
                                        yp[:], wo8_sb[:, :, ms], oh8p[:],
                                        start=True, stop=False,
                                        perf_mode=DR, skip_group_check=True)
                                    nc.tensor.matmul(
                                        yp[:], wo16_sb[:, 2, ms], oh16[:],
                                        start=False, stop=True,
                                        skip_group_check=True)
                                else:
                                    for h in range(QH):
                                        nc.tensor.matmul(
                                            yp[:], wo16_sb[:, h, ms],
                                            oh_dsts[h],
                                            start=(h == 0),
                                            stop=(h == QH - 1))
                                col = mt % 12
                                if mt % 2 == 0:
                                    nc.vector.tensor_copy(y_all[:, col, :],
                                                          yp[:])
                                else:
                                    nc.scalar.copy(y_all[:, col, :], yp[:])
                                if mt in (5, 11, 17, 23):
                                    lo = {5: 0, 11: 6, 17: 12, 23: 18}[mt]
                                    nc.sync.dma_start(
                                        y_ap[b][:, lo:mt + 1, qs],
                                        y_all[:, lo % 12:col + 1, :])

                        # sparse qc0 between the dense qc2/qc3 phases. Every
                        # qc's wo-loop is emitted one attention phase late so
                        # the softmax normalization (ACT+DVE tail, behind the
                        # previous wo's y-copies in the DVE queue) is
                        # finished before the PE reaches the wo matmuls.
                        deferred_wo = None
                        for qc in (1, 2, 0, 3):
                            qs = slice(qc * SC, (qc + 1) * SC)
                            wo_dr = WO_FP8 and qc != 0
                            if wo_dr:
                                oh8p = asb.tile([128, 2, SC], F8,
                                                tag="oh8", bufs=3,
                                                name="oh8p")
                                oh16 = asb.tile([128, SC], F16,
                                                tag="oh16", bufs=6,
                                                name="oh16")
                                oh_dsts = [oh8p[:, 0, :], oh8p[:, 1, :],
                                           oh16[:]]
                            else:
                                oh_dsts = [asb.tile([128, SC], F16,
                                                    tag="oh16", bufs=6,
                                                    name="oh")[:]
                                           for _ in range(QH)]
                            pending = None
                            for h in range(QH):
                                av = aps.tile([128, SC], F32, tag="av",
                                              bufs=3, name="av")
                                r = aps.tile([1, SC], F32, tag="r", bufs=1,
                                             name="r")
                                # full (unmasked) kt pairs: kts 0..4qc-1, fp8 DR
                                for p in range(2 * qc):
                                    s2 = aps.tile([128, 2 * SC], F32,
                                                  tag="sp", bufs=2, name="s2")
                                    for half in (0, 1):
                                        kt = 2 * p + half
                                        nc.tensor.matmul(
                                            s2[:, half * SC:(half + 1) * SC],
                                            K_sb[:, kt * 128:(kt + 1) * 128],
                                            Q_sb[h][:, qs],
                                            start=True, stop=True)
                                    if pending is not None:
                                        norm_pe(pending)
                                        pending = None
                                    e2 = asb.tile([128, 2, SC], F8, tag="e8",
                                                  bufs=4, name="e2")
                                    nc.scalar.activation(
                                        e2[:].rearrange("p a s -> p (a s)"),
                                        s2[:], AFT.Exp,
                                        scale=SCALE_EXP, bias=ebias_sb[:])
                                    st = (p == 0)
                                    nc.tensor.matmul(
                                        av[:], V8[:, 2 * p:2 * p + 2, :],
                                        e2[:], start=st, stop=False,
                                        perf_mode=DR, skip_group_check=True)
                                    nc.tensor.matmul(
                                        r[:], ones8_sb[:, :, 0:1], e2[:],
                                        start=st, stop=False,
                                        perf_mode=DR, skip_group_check=True)

                                # diagonal kts 4qc..4qc+3: fp16, col-trimmed,
                                # two kts packed contiguously per psum pair
                                # tile so one exp covers both.
                                sD = eD2 = None
                                for i in range(4):
                                    kt = 4 * qc + i
                                    thr = 128 * i
                                    w = SC - thr
                                    if i % 2 == 0:
                                        sD = aps.tile([128, 2 * SC], F32,
                                                      tag="sp", bufs=2,
                                                      name="sD")
                                        eD2 = asb.tile([128, 2 * SC], F16,
                                                       tag="e16", bufs=4,
                                                       name="eD2")
                                        base = 0
                                    else:
                                        base = SC - 128 * (i - 1)  # w_even
                                    nc.tensor.matmul(
                                        sD[:, base:base + w],
                                        K_sb[:, kt * 128:(kt + 1) * 128],
                                        Q_sb[h][:, qc * SC + thr:
                                                (qc + 1) * SC],
                                        start=True, stop=True)
                                    if pending is not None:
                                        norm_pe(pending)
                                        pending = None
                                    if i % 2 == 1:
                                        # one exp over both packed regions
                                        nc.scalar.activation(
                                            eD2[:, 0:base + w],
                                            sD[:, 0:base + w], AFT.Exp,
                                            scale=SCALE_EXP, bias=ebias_sb[:])
                                    for ii in (i - 1, i) if i % 2 else ():
                                        tt = 128 * ii
                                        bb = 0 if ii % 2 == 0 else base
                                        ww = SC - tt
                                        nc.vector.tensor_mul(
                                            eD2[:, bb:bb + 128],
                                            eD2[:, bb:bb + 128], tri_sb[:])
                                        st = (qc == 0 and ii == 0)
                                        sp_ = (ii == 3)
                                        nc.tensor.matmul(
                                            av[:, tt:SC],
                                            V16[:, 4 * qc + ii, :],
                                            eD2[:, bb:bb + ww],
                                            start=st, stop=sp_,
                                            skip_group_check=True)
                                        nc.tensor.matmul(
                                            r[:, tt:SC], ones16_sb[:],
                                            eD2[:, bb:bb + ww],
                                            start=st, stop=sp_,
                                            skip_group_check=True)

                                # softmax normalization: DVE part now, PE
                                # part deferred into the next head
                                inv = asb.tile([1, SC], F32, tag="inv",
                                               bufs=2, name="inv")
                                nc.vector.reciprocal_approx_fast(inv[:], r[:])
                                inv16 = asb.tile([1, SC], F16, tag="inv16",
                                                 bufs=2, name="inv16")
                                nc.vector.tensor_copy(inv16[:], inv[:])
                                pending = (av, inv16, oh_dsts[h])
                            norm_pe(pending)

                            wo_args = (qs, wo_dr,
                                       oh8p if wo_dr else None,
                                       oh16 if wo_dr else None, oh_dsts)
                            if deferred_wo is not None:
                                emit_wo(*deferred_wo)
                            deferred_wo = wo_args
                        emit_wo(*deferred_wo)

    nc.compile()
    return nc


def make_inputs(x, freqs_cos, freqs_sin, mask, wq, wk, wv, wo):
    """Host-side preprocessing -> per-core input maps."""
    f32, f16 = np.float32, np.float16
    f8 = ml_dtypes.float8_e4m3

    x = np.asarray(x, f32)
    xs = (x * SX).reshape(B, NSC, SC, CT, 128)
    # chunk-major [b, chunk, partition, ct, s]: 3KB+ contiguous DRAM per
    # partition per chunk sub-DMA
    xT = np.ascontiguousarray(np.transpose(xs, (0, 1, 4, 3, 2)).astype(f8))
    xT16 = np.ascontiguousarray(
        np.transpose(xs[:, 0], (0, 3, 2, 1)).astype(f16))
    cosT = np.ascontiguousarray(
        np.concatenate([freqs_cos, freqs_cos], axis=1).T.astype(f16))
    sinT = np.concatenate([freqs_sin, freqs_sin], axis=1).T.astype(f32).copy()
    sinT[:HD // 2] *= -1.0  # sign of rotate-half folded in
    sinT = np.ascontiguousarray(sinT.astype(f16))

    # sanity: mask must be the causal tril mask the kernel hardcodes
    m = np.asarray(mask, f32)[0, 0]
    assert (m[np.tril_indices(4)] == 0).all() and m[0, 1] < -1e8, "non-causal mask"

    tri = np.ascontiguousarray(
        (np.arange(128)[None, :] >= np.arange(128)[:, None]).astype(f16))
    identity = np.ascontiguousarray(np.eye(128, dtype=f16))

    wqTs = np.asarray(wq, f32).T * SW
    wkTs = np.asarray(wk, f32).T * SW
    wvTs = np.asarray(wv, f32).T * SW
    wqT, wqT16 = wqTs.astype(f8), wqTs.astype(f16)
    wkT, wkT16 = wkTs.astype(f8), wkTs.astype(f16)
    wvT, wvT16 = wvTs.astype(f8), wvTs.astype(f16)
    woT = np.asarray(wo, f32).T * SW

    in_maps = []
    for h in range(N_CORES):
        qsl = slice(h * QH * HD, (h + 1) * QH * HD)
        ksl = slice(h * HD, (h + 1) * HD)
        im = {
            "xT": xT,
            "xT16": xT16,
            "cosT": cosT,
            "sinT": sinT,
            "wq": np.ascontiguousarray(wqT[:, qsl]),
            "wk": np.ascontiguousarray(wkT[:, ksl]),
            "wv": np.ascontiguousarray(wvT[:, ksl]),
            "wq16": np.ascontiguousarray(wqT16[:, qsl]),
            "wk16": np.ascontiguousarray(wkT16[:, ksl]),
            "wv16": np.ascontiguousarray(wvT16[:, ksl]),
            "tri": tri,
            "ident": identity,
            "ones8": np.ones((128, 2, 16), f8),
            "ones16": np.ones((128, 1), f16),
            "onesr": np.ones((1, 128), f16),
            "ebias": np.full((128, 1), EXP_BIAS, f32),
        }
        wo_core = woT[qsl, :]
        im["wo8"] = np.ascontiguousarray(wo_core[:2 * HD, :]).astype(f8)
        im["wo16"] = np.ascontiguousarray(wo_core).astype(f16)
        in_maps.append(im)
    return in_maps


_CACHE = {}


def kernel(x, freqs_cos, freqs_sin, mask, wq, wk, wv, wo):
    global LAST_EXEC_NS, LAST_RESULTS
    assert tuple(x.shape) == (B, S, D), x.shape

    in_maps = make_inputs(x, freqs_cos, freqs_sin, mask, wq, wk, wv, wo)

    if "nc" not in _CACHE:
        _CACHE["nc"] = build_program()
    nc = _CACHE["nc"]

    kwargs = {}
    if TRACE:
        kwargs = dict(trace=True, trace_cores=[0])
    res = run_bass_kernel_spmd(nc, in_maps, list(range(N_CORES)), **kwargs)
    LAST_EXEC_NS = res.exec_time_ns
    LAST_RESULTS = res

    acc = np.zeros((B, D, S), np.float32)
    for i in range(N_CORES):
        acc += res.results[i]["yT"].astype(np.float32)
    acc *= 1.0 / (SW * SV)
    y = np.ascontiguousarray(np.transpose(acc, (0, 2, 1)))
    return y



# revision 67
# speedup vs baseline: 1.2144x; 1.0262x over previous
"""GQA attention kernel for Trainium2 (8 NeuronCores, Bass/Tile).

Problem: B=2, S=2048, D=3072, 24 Q heads / 8 KV heads, HD=128, RoPE,
additive causal mask, softmax, output projection.

Sharding: tensor-parallel over heads. Core h owns KV head h and Q heads
{3h, 3h+1, 3h+2} for BOTH batch elements. Each core produces a partial
y^T = wo_slice^T.T @ attn_out_heads^T of shape (B, D, S) in fp16; the
host sums the 8 partials in fp32 and transposes back.

Layout: everything stays transposed ([feature, token]) on chip so every
matmul contracts on the partition dim with a 512-wide fp16 moving
operand (1 cycle/row on the PE).

Key optimizations over the fp16 baseline:
  - attention probabilities e=exp(score*scale-3) and V are fp8e4 for
    fully-unmasked causal blocks, letting attn@V and the softmax row-sum
    matmuls use DoubleRow perf mode (two 128-row k-tiles per
    instruction, ~2x). Diagonal blocks stay fp16 so every row's softmax
    denominator is exact enough and can never flush to zero.
  - causal diagonal blocks are column-trimmed: scores/exp/attn@V only
    touch the valid triangle's column range; the triangle itself is
    masked with one shared 128x128 lower-triangular multiplier.
  - bulk DMA: one descriptor per x chunk / weight tensor / y chunk
    (~40 DMA issues total vs ~570), fp16 y partials.
  - RoPE entirely on the vector engine: rotate-half is expressed with
    cross-partition-base operand slices, no SBUF-SBUF DMAs and no
    scalar-engine copies; sin sign folded in on the host.
  - exp batched 2 causal blocks per activation instruction (reads a
    [128,1024] 2-bank PSUM window).
"""

import math
import os
import sys

import numpy as np

for _p in ("/opt/trn_rl_repo",):
    if os.path.isdir(_p) and _p not in sys.path:
        sys.path.insert(0, _p)

import ml_dtypes  # noqa: E402

import concourse.bass as bass  # noqa: E402
import concourse.mybir as mybir  # noqa: E402
import concourse.tile as tile  # noqa: E402
from concourse import bacc  # noqa: E402
from concourse.bass_utils import run_bass_kernel_spmd  # noqa: E402

F32 = mybir.dt.float32
F16 = mybir.dt.float16
F8 = mybir.dt.float8e4
AFT = mybir.ActivationFunctionType
DR = mybir.MatmulPerfMode.DoubleRow

N_CORES = 8

# Set by test harness to capture a profile on the next kernel() call.
TRACE = False
LAST_EXEC_NS = None
LAST_RESULTS = None

B, S, D = 2, 2048, 3072
QH, HD, SC = 3, 128, 512
CT = D // 128          # 24 contraction tiles for projections
KT = S // 128          # 16 key tiles
NSC = S // SC          # 4 token chunks
PB = SC // 128         # 4 key tiles per chunk
SCALE = 1.0 / math.sqrt(HD)
EXP_BIAS = -3.0        # uniform; cancels in softmax normalization

# fp8 prescales: wq/wk/wv/wo would be subnormal in e4m3 at their native
# 0.02 sigma, so weights carry x64 and activations x16 on chip. The x/w
# product scale (1024) is divided back out inside RoPE's cos/sin tables;
# V carries x16 via the transpose identity (I * 16/1024) so the
# attention output oh is x16, matching the fp8 wo path whose product
# scale (64*16 = 1024) the host divides out of the final reduction.
SX = 16.0              # x -> fp8
SW = 64.0              # wq/wk/wv/wo -> fp8 (wo16 also x64 to match)
SV = 16.0              # V (and hence oh) on-chip scale
# Q/K ride at SX*SW x true scale in fp16 (cos/sin stay full scale to
# dodge fp16 subnormals); exp's scale arg absorbs the dequant for free.
SCALE_EXP = SCALE / (SX * SW) ** 2
WO_FP8 = True          # heads 0,1 of wo in fp8 DoubleRow; head 2 fp16


def build_program():
    nc = bacc.Bacc("TRN2", target_bir_lowering=False, debug=False,
                   num_devices=N_CORES)

    # x is stored chunk-major [b, chunk, partition, ct, s] so each chunk
    # DMA reads 3KB+ contiguous DRAM per partition (512B segments from a
    # [B,D,S] layout ran at ~half DMA rate and stalled the QKV DR chain).
    xT = nc.declare_dram_parameter("xT", [B, NSC, 128, CT, SC], F8,
                                   isOutput=False)
    # tokens 0..SC-1 in fp16: chunk 0's projections run in fp16 so the
    # short causal rows (few softmax terms, no error averaging) stay
    # accurate; fp8 noise there would blow the early-row error up 5x.
    xT16 = nc.declare_dram_parameter("xT16", [B, 128, CT, SC], F16,
                                     isOutput=False)
    cosT = nc.declare_dram_parameter("cosT", [HD, S], F16, isOutput=False)
    sinT = nc.declare_dram_parameter("sinT", [HD, S], F16, isOutput=False)
    # all weights are pre-permuted on the host into their exact SBUF
    # layouts ([partition, ...]) so every weight DMA reads long
    # contiguous DRAM runs (the "(ct p) f" gather pattern had 256-768B
    # segments and ran at half DMA rate, serializing startup).
    wq = nc.declare_dram_parameter("wq", [128, CT, QH * HD], F8,
                                   isOutput=False)
    wk = nc.declare_dram_parameter("wk", [128, CT, HD], F8, isOutput=False)
    wv = nc.declare_dram_parameter("wv", [128, CT, HD], F8, isOutput=False)
    wq16 = nc.declare_dram_parameter("wq16", [128, CT, QH * HD], F16,
                                     isOutput=False)
    wk16 = nc.declare_dram_parameter("wk16", [128, CT, HD], F16,
                                     isOutput=False)
    wv16 = nc.declare_dram_parameter("wv16", [128, CT, HD], F16,
                                     isOutput=False)
    # wo in both precisions: fp8 DoubleRow pair (heads 0,1) for the
    # diffuse-attention query chunks, full fp16 for qc0 whose
    # concentrated rows have large |oh| and can't absorb fp8 noise.
    wo8 = nc.declare_dram_parameter("wo8", [128, 2, D], F8, isOutput=False)
    wo16 = nc.declare_dram_parameter("wo16", [128, QH, D], F16,
                                     isOutput=False)
    tri = nc.declare_dram_parameter("tri", [128, 128], F16, isOutput=False)
    ident = nc.declare_dram_parameter("ident", [128, 128], F16, isOutput=False)
    ones8 = nc.declare_dram_parameter("ones8", [128, 2, 16], F8, isOutput=False)
    ones16 = nc.declare_dram_parameter("ones16", [128, 1], F16, isOutput=False)
    onesr = nc.declare_dram_parameter("onesr", [1, 128], F16, isOutput=False)
    ebias = nc.declare_dram_parameter("ebias", [128, 1], F32, isOutput=False)
    yT = nc.declare_dram_parameter("yT", [B, D, S], F16, isOutput=True)

    x_ap = xT.ap()
    x16_ap = xT16.ap()
    y_ap = yT.ap().rearrange("b (ct p) s -> b p ct s", p=128)

    with tile.TileContext(nc) as tc:
        from contextlib import ExitStack
        with ExitStack() as top:
            const = top.enter_context(tc.tile_pool(name="const", bufs=1))
            stream = top.enter_context(tc.tile_pool(name="stream", bufs=1))

            wq_sb = const.tile([128, CT, QH * HD], F8, name="wq_sb")
            wk_sb = const.tile([128, CT, HD], F8, name="wk_sb")
            wv_sb = const.tile([128, CT, HD], F8, name="wv_sb")
            wq16_sb = const.tile([128, CT, QH * HD], F16, name="wq16_sb")
            wk16_sb = const.tile([128, CT, HD], F16, name="wk16_sb")
            wv16_sb = const.tile([128, CT, HD], F16, name="wv16_sb")
            wo8_sb = const.tile([128, 2, D], F8, name="wo8_sb")
            wo16_sb = const.tile([128, QH, D], F16, name="wo16_sb")
            cos_sb = const.tile([128, S], F16, name="cos_sb")
            sin_sb = const.tile([128, S], F16, name="sin_sb")
            tri_sb = const.tile([128, 128], F16, name="tri_sb")
            ident_sb = const.tile([128, 128], F16, name="ident_sb")
            # [128, 2, 16] so the DoubleRow weights AP subtile step is
            # 16B-aligned (s3_lw_dual_fp8_restrictions); only col 0 is used.
            ones8_sb = const.tile([128, 2, 16], F8, name="ones8_sb")
            ones16_sb = const.tile([128, 1], F16, name="ones16_sb")
            onesr_sb = const.tile([1, 128], F16, name="onesr_sb")
            ebias_sb = const.tile([128, 1], F32, name="ebias_sb")

            # xt chunk prefetcher: one-chunk lookahead, 4 sub-DMAs per chunk
            # so the first matmuls can start before the whole chunk lands.
            xt_tiles = {}

            def ensure_xt(b, sc):
                if sc >= NSC or (b, sc) in xt_tiles:
                    return
                # four independent 6-ct sub-tiles per chunk: tile-level DMA
                # dependencies let the first matmuls start after the first
                # quarter lands instead of waiting for the whole chunk.
                subs = []
                for c in range(0, CT, 6):
                    if sc == 0:
                        t = stream.tile([128, 6, SC], F16, tag="xt16",
                                        bufs=4, name="xt16")
                        nc.sync.dma_start(t[:], x16_ap[b][:, c:c + 6, :])
                    else:
                        t = stream.tile([128, 6, SC], F8, tag="xt", bufs=8,
                                        name="xt")
                        nc.sync.dma_start(t[:], x_ap[b][sc][:, c:c + 6, :])
                    subs.append(t)
                xt_tiles[(b, sc)] = subs

            # DMA issue order: single sync HWDGE ring, sequenced so each
            # transfer lands just before its first consumer needs it.
            # The fp8 chunk-1 path starts first (small transfers) while the
            # 7MB of fp16 chunk-0 weights/x stream in behind it.
            nc.sync.dma_start(wk_sb[:], wk.ap())
            ensure_xt(0, 1)
            nc.sync.dma_start(wv_sb[:], wv.ap())
            nc.sync.dma_start(wq_sb[:], wq.ap())
            nc.sync.dma_start(cos_sb[:], cosT.ap())
            nc.sync.dma_start(sin_sb[:], sinT.ap())
            nc.sync.dma_start(wk16_sb[:], wk16.ap())
            ensure_xt(0, 0)
            nc.sync.dma_start(wv16_sb[:], wv16.ap())
            nc.sync.dma_start(wq16_sb[:], wq16.ap())
            nc.sync.dma_start(tri_sb[:], tri.ap())
            nc.sync.dma_start(ident_sb[:], ident.ap())
            nc.sync.dma_start(ones8_sb[:], ones8.ap())
            nc.sync.dma_start(ones16_sb[:], ones16.ap())
            nc.sync.dma_start(onesr_sb[:], onesr.ap())
            nc.sync.dma_start(ebias_sb[:], ebias.ap())
            wo_loaded = [False]

            def load_wo():
                if not wo_loaded[0]:
                    wo_loaded[0] = True
                    nc.sync.dma_start(wo8_sb[:], wo8.ap())
                    nc.sync.dma_start(wo16_sb[:], wo16.ap())

            for b in range(B):
                with ExitStack() as bctx:
                    bpool = bctx.enter_context(
                        tc.tile_pool(name=f"b{b}_persist", bufs=1))
                    K_sb = bpool.tile([128, S], F16, name=f"K_sb{b}")
                    V16 = bpool.tile([128, KT, 128], F16, name=f"V16_{b}")
                    V8 = bpool.tile([128, KT, 128], F8, name=f"V8_{b}")
                    Q_sb = [bpool.tile([128, S], F16, name=f"Q_sb{b}_{h}")
                            for h in range(QH)]

                    # ---------------- QKV projection + RoPE ----------------
                    with ExitStack() as pctx:
                        pps = pctx.enter_context(
                            tc.tile_pool(name=f"b{b}_qkv_ps", bufs=1,
                                         space="PSUM"))
                        sp = pctx.enter_context(
                            tc.tile_pool(name=f"b{b}_qkv_sb", bufs=1))

                        def rope(acc, dst_slice, cs):
                            # stage the fp32 PSUM acc to fp16 on the (idle)
                            # scalar engine: the DVE muls then run in 2x
                            # 16-bit mode (~345ns vs ~690ns fp32-read) and
                            # the PSUM bank frees a group earlier.
                            a16 = sp.tile([128, SC], F16, tag="a16", bufs=3,
                                          name="a16")
                            nc.scalar.copy(a16[:], acc[:])
                            tmp1 = sp.tile([128, SC], F16, tag="t1", bufs=3,
                                           name="tmp1")
                            nc.vector.tensor_mul(tmp1[:], a16[:],
                                                 cos_sb[:, cs])
                            # sin table is half-rolled on the host so both
                            # SBUF inputs of each mul share a base partition
                            tmp2 = sp.tile([128, SC], F16, tag="t2", bufs=3,
                                           name="tmp2")
                            nc.vector.tensor_mul(tmp2[0:64, :],
                                                 a16[64:128, :],
                                                 sin_sb[64:128, cs])
                            nc.vector.tensor_mul(tmp2[64:128, :],
                                                 a16[0:64, :],
                                                 sin_sb[0:64, cs])
                            nc.vector.tensor_add(dst_slice, tmp1[:], tmp2[:])

                        # chunk 1 (fp8, small DMA) first so the PE starts
                        # ~1us in; chunk 0's 7MB of fp16 streams in behind.
                        SCO = (1, 0, 2, 3)
                        for si, sc in enumerate(SCO):
                            cs = slice(sc * SC, (sc + 1) * SC)
                            ensure_xt(b, sc)
                            xt = xt_tiles.pop((b, sc))
                            if si + 1 < NSC:
                                ensure_xt(b, SCO[si + 1])

                            # groups streamed one at a time: K, V, Q0..Q2
                            if sc == 0:
                                groups = [("k", wk16_sb, 0),
                                          ("v", wv16_sb, 0),
                                          ("q0", wq16_sb, 0),
                                          ("q1", wq16_sb, 1),
                                          ("q2", wq16_sb, 2)]
                            else:
                                groups = [("k", wk_sb, 0), ("v", wv_sb, 0),
                                          ("q0", wq_sb, 0),
                                          ("q1", wq_sb, 1),
                                          ("q2", wq_sb, 2)]
                            for gname, wsb, gi in groups:
                                acc = pps.tile([128, SC], F32, tag="acc",
                                               bufs=5, name="acc")
                                fs = slice(gi * HD, (gi + 1) * HD)
                                if sc == 0:
                                    for ct in range(CT):
                                        nc.tensor.matmul(
                                            acc[:], wsb[:, ct, fs],
                                            xt[ct // 6][:, ct % 6, :],
                                            start=(ct == 0),
                                            stop=(ct == CT - 1))
                                else:
                                    for cp in range(CT // 2):
                                        c0 = (2 * cp) % 6
                                        nc.tensor.matmul(
                                            acc[:],
                                            wsb[:, 2 * cp:2 * cp + 2, fs],
                                            xt[cp // 3][:, c0:c0 + 2, :],
                                            start=(cp == 0),
                                            stop=(cp == CT // 2 - 1),
                                            perf_mode=DR)
                                if gname == "k":
                                    rope(acc, K_sb[:, cs], cs)
                                elif gname == "v":
                                    vstage = sp.tile([128, SC], F16,
                                                     tag="vst", bufs=2,
                                                     name="vstage")
                                    # dequant x/w prescales and apply the
                                    # on-chip V scale (transpose ignores the
                                    # identity's values, so scale here)
                                    nc.scalar.activation(
                                        vstage[:], acc[:], AFT.Copy,
                                        scale=SV / (SX * SW))
                                    vps = pps.tile([128, PB, 128], F16,
                                                   tag="vtr", bufs=2,
                                                   name="vps")
                                    for j in range(PB):
                                        nc.tensor.transpose(
                                            vps[:, j, :],
                                            vstage[:, j * 128:(j + 1) * 128],
                                            ident_sb[:])
                                    ks = slice(sc * PB, (sc + 1) * PB)
                                    nc.vector.tensor_copy(V16[:, ks, :],
                                                          vps[:])
                                    # fp8 V copy on the scalar engine: DVE
                                    # is the QKV phase's critical tail
                                    nc.scalar.copy(V8[:, ks, :], vps[:])
                                else:
                                    h = int(gname[1])
                                    rope(acc, Q_sb[h][:, cs], cs)

                    # ---------------- attention + out-projection ----------------
                    with ExitStack() as actx:
                        aps = actx.enter_context(
                            tc.tile_pool(name=f"b{b}_attn_ps", bufs=1,
                                         space="PSUM"))
                        asb = actx.enter_context(
                            tc.tile_pool(name=f"b{b}_attn_sb", bufs=1))

                        # wo first: it is needed ~25us into this attention
                        # phase; the next batch's x only at the next QKV.
                        load_wo()
                        if b + 1 < B:
                            ensure_xt(b + 1, 1)
                            ensure_xt(b + 1, 0)

                        def norm_pe(st):
                            """PE/ACT/DVE tail of softmax normalization.

                            Emitted *after* the next head's first score
                            matmuls so the broadcast matmul never stalls the
                            PE on the reciprocal chain."""
                            av, inv16, oh = st
                            invb = aps.tile([128, SC], F32, tag="av",
                                            bufs=3, name="invb")
                            nc.tensor.matmul(invb[:], onesr_sb[:],
                                             inv16[:], start=True, stop=True)
                            invb_sb = asb.tile([128, SC], F32, tag="invbsb",
                                               bufs=2, name="invb_sb")
                            nc.scalar.copy(invb_sb[:], invb[:])
                            nc.vector.tensor_mul(oh, av[:], invb_sb[:])

                        def emit_wo(qs, wo_dr, oh8p, oh16, oh_dsts):
                            # out-projection; y tiles rotate through the av
                            # psum tag so score-pair banks stay free for the
                            # next qc's attention. y stages through two
                            # 12-column half tiles to halve SBUF footprint.
                            y_all = None
                            for mt in range(CT):
                                if mt % 12 == 0:
                                    y_all = stream.tile([128, 12, SC], F16,
                                                        tag="yall", bufs=2,
                                                        name="y_half")
                                yp = aps.tile([128, SC], F32, tag="av",
                                              bufs=3, name="yp")
                                ms = slice(mt * 128, (mt + 1) * 128)
                                if wo_dr:
                                    nc.tensor.matmul(
                                        yp[:], wo8_sb[:, :, ms], oh8p[:],
                                        start=True, stop=False,
                                        perf_mode=DR, skip_group_check=True)
                                    nc.tensor.matmul(
                                        yp[:], wo16_sb[:, 2, ms], oh16[:],
                                        start=False, stop=True,
                                        skip_group_check=True)
                                else:
                                    for h in range(QH):
                                        nc.tensor.matmul(
                                            yp[:], wo16_sb[:, h, ms],
                                            oh_dsts[h],
                                            start=(h == 0),
                                            stop=(h == QH - 1))
                                col = mt % 12
                                if mt % 2 == 0:
                                    nc.vector.tensor_copy(y_all[:, col, :],
                                                          yp[:])
                                else:
                                    nc.scalar.copy(y_all[:, col, :], yp[:])
                                if mt in (5, 11, 17, 23):
                                    lo = {5: 0, 11: 6, 17: 12, 23: 18}[mt]
                                    nc.sync.dma_start(
                                        y_ap[b][:, lo:mt + 1, qs],
                                        y_all[:, lo % 12:col + 1, :])

                        # sparse qc0 between the dense qc2/qc3 phases. Every
                        # qc's wo-loop is emitted one attention phase late so
                        # the softmax normalization (ACT+DVE tail, behind the
                        # previous wo's y-copies in the DVE queue) is
                        # finished before the PE reaches the wo matmuls.
                        deferred_wo = None
                        for qc in (1, 2, 0, 3):
                            qs = slice(qc * SC, (qc + 1) * SC)
                            wo_dr = WO_FP8 and qc != 0
                            if wo_dr:
                                oh8p = asb.tile([128, 2, SC], F8,
                                                tag="oh8", bufs=3,
                                                name="oh8p")
                                oh16 = asb.tile([128, SC], F16,
                                                tag="oh16", bufs=6,
                                                name="oh16")
                                oh_dsts = [oh8p[:, 0, :], oh8p[:, 1, :],
                                           oh16[:]]
                            else:
                                oh_dsts = [asb.tile([128, SC], F16,
                                                    tag="oh16", bufs=6,
                                                    name="oh")[:]
                                           for _ in range(QH)]
                            pending = None
                            for h in range(QH):
                                av = aps.tile([128, SC], F32, tag="av",
                                              bufs=3, name="av")
                                r = aps.tile([1, SC], F32, tag="r", bufs=1,
                                             name="r")
                                # full (unmasked) kt pairs: kts 0..4qc-1, fp8 DR
                                for p in range(2 * qc):
                                    s2 = aps.tile([128, 2 * SC], F32,
                                                  tag="sp", bufs=2, name="s2")
                                    for half in (0, 1):
                                        kt = 2 * p + half
                                        nc.tensor.matmul(
                                            s2[:, half * SC:(half + 1) * SC],
                                            K_sb[:, kt * 128:(kt + 1) * 128],
                                            Q_sb[h][:, qs],
                                            start=True, stop=True)
                                    if pending is not None:
                                        norm_pe(pending)
                                        pending = None
                                    e2 = asb.tile([128, 2, SC], F8, tag="e8",
                                                  bufs=4, name="e2")
                                    nc.scalar.activation(
                                        e2[:].rearrange("p a s -> p (a s)"),
                                        s2[:], AFT.Exp,
                                        scale=SCALE_EXP, bias=ebias_sb[:])
                                    st = (p == 0)
                                    nc.tensor.matmul(
                                        av[:], V8[:, 2 * p:2 * p + 2, :],
                                        e2[:], start=st, stop=False,
                                        perf_mode=DR, skip_group_check=True)
                                    nc.tensor.matmul(
                                        r[:], ones8_sb[:, :, 0:1], e2[:],
                                        start=st, stop=False,
                                        perf_mode=DR, skip_group_check=True)

                                # diagonal kts 4qc..4qc+3: fp16, col-trimmed,
                                # two kts packed contiguously per psum pair
                                # tile so one exp covers both.
                                sD = eD2 = None
                                for i in range(4):
                                    kt = 4 * qc + i
                                    thr = 128 * i
                                    w = SC - thr
                                    if i % 2 == 0:
                                        sD = aps.tile([128, 2 * SC], F32,
                                                      tag="sp", bufs=2,
                                                      name="sD")
                                        eD2 = asb.tile([128, 2 * SC], F16,
                                                       tag="e16", bufs=4,
                                                       name="eD2")
                                        base = 0
                                    else:
                                        base = SC - 128 * (i - 1)  # w_even
                                    nc.tensor.matmul(
                                        sD[:, base:base + w],
                                        K_sb[:, kt * 128:(kt + 1) * 128],
                                        Q_sb[h][:, qc * SC + thr:
                                                (qc + 1) * SC],
                                        start=True, stop=True)
                                    if pending is not None:
                                        norm_pe(pending)
                                        pending = None
                                    if i % 2 == 1:
                                        # one exp over both packed regions
                                        nc.scalar.activation(
                                            eD2[:, 0:base + w],
                                            sD[:, 0:base + w], AFT.Exp,
                                            scale=SCALE_EXP, bias=ebias_sb[:])
                                    for ii in (i - 1, i) if i % 2 else ():
                                        tt = 128 * ii
                                        bb = 0 if ii % 2 == 0 else base
                                        ww = SC - tt
                                        nc.vector.tensor_mul(
                                            eD2[:, bb:bb + 128],
                                            eD2[:, bb:bb + 128], tri_sb[:])
                                        st = (qc == 0 and ii == 0)
                                        sp_ = (ii == 3)
                                        nc.tensor.matmul(
                                            av[:, tt:SC],
                                            V16[:, 4 * qc + ii, :],
                                            eD2[:, bb:bb + ww],
                                            start=st, stop=sp_,
                                            skip_group_check=True)
                                        nc.tensor.matmul(
                                            r[:, tt:SC], ones16_sb[:],
                                            eD2[:, bb:bb + ww],
                                            start=st, stop=sp_,
                                            skip_group_check=True)

                                # softmax normalization: DVE part now, PE
                                # part deferred into the next head
                                inv = asb.tile([1, SC], F32, tag="inv",
                                               bufs=2, name="inv")
                                nc.vector.reciprocal_approx_fast(inv[:], r[:])
                                inv16 = asb.tile([1, SC], F16, tag="inv16",
                                                 bufs=2, name="inv16")
                                nc.vector.tensor_copy(inv16[:], inv[:])
                                pending = (av, inv16, oh_dsts[h])
                            norm_pe(pending)

                            wo_args = (qs, wo_dr,
                                       oh8p if wo_dr else None,
                                       oh16 if wo_dr else None, oh_dsts)
                            if deferred_wo is not None:
                                emit_wo(*deferred_wo)
                            deferred_wo = wo_args
                        emit_wo(*deferred_wo)

    nc.compile()
    return nc


def make_inputs(x, freqs_cos, freqs_sin, mask, wq, wk, wv, wo):
    """Host-side preprocessing -> per-core input maps."""
    f32, f16 = np.float32, np.float16
    f8 = ml_dtypes.float8_e4m3

    x = np.asarray(x, f32)
    xs = (x * SX).reshape(B, NSC, SC, CT, 128)
    # chunk-major [b, chunk, partition, ct, s]: 3KB+ contiguous DRAM per
    # partition per chunk sub-DMA
    xT = np.ascontiguousarray(np.transpose(xs, (0, 1, 4, 3, 2)).astype(f8))
    xT16 = np.ascontiguousarray(
        np.transpose(xs[:, 0], (0, 3, 2, 1)).astype(f16))
    cosT = np.ascontiguousarray(
        np.concatenate([freqs_cos, freqs_cos], axis=1).T.astype(f16))
    sinT = np.concatenate([freqs_sin, freqs_sin], axis=1).T.astype(f32).copy()
    sinT[:HD // 2] *= -1.0  # sign of rotate-half folded in
    # half-rolled: row p holds the sin the rope mul with base partition p
    # reads (keeps both SBUF operands of each DVE mul on one base)
    sinT = np.roll(sinT, HD // 2, axis=0)
    sinT = np.ascontiguousarray(sinT.astype(f16))

    # sanity: mask must be the causal tril mask the kernel hardcodes
    m = np.asarray(mask, f32)[0, 0]
    assert (m[np.tril_indices(4)] == 0).all() and m[0, 1] < -1e8, "non-causal mask"

    tri = np.ascontiguousarray(
        (np.arange(128)[None, :] >= np.arange(128)[:, None]).astype(f16))
    identity = np.ascontiguousarray(np.eye(128, dtype=f16))

    wqTs = np.asarray(wq, f32).T * SW
    wkTs = np.asarray(wk, f32).T * SW
    wvTs = np.asarray(wv, f32).T * SW
    wqT, wqT16 = wqTs.astype(f8), wqTs.astype(f16)
    wkT, wkT16 = wkTs.astype(f8), wkTs.astype(f16)
    wvT, wvT16 = wvTs.astype(f8), wvTs.astype(f16)
    woT = np.asarray(wo, f32).T * SW

    def sbuf_w(a):
        # [D, F] (or [G*128, F]) -> [128, G, F]: the exact SBUF layout, so
        # the weight DMA reads one long contiguous run per partition
        g = a.shape[0] // 128
        return np.ascontiguousarray(a.reshape(g, 128, -1).transpose(1, 0, 2))

    in_maps = []
    for h in range(N_CORES):
        qsl = slice(h * QH * HD, (h + 1) * QH * HD)
        ksl = slice(h * HD, (h + 1) * HD)
        im = {
            "xT": xT,
            "xT16": xT16,
            "cosT": cosT,
            "sinT": sinT,
            "wq": sbuf_w(wqT[:, qsl]),
            "wk": sbuf_w(wkT[:, ksl]),
            "wv": sbuf_w(wvT[:, ksl]),
            "wq16": sbuf_w(wqT16[:, qsl]),
            "wk16": sbuf_w(wkT16[:, ksl]),
            "wv16": sbuf_w(wvT16[:, ksl]),
            "tri": tri,
            "ident": identity,
            "ones8": np.ones((128, 2, 16), f8),
            "ones16": np.ones((128, 1), f16),
            "onesr": np.ones((1, 128), f16),
            "ebias": np.full((128, 1), EXP_BIAS, f32),
        }
        wo_core = woT[qsl, :]
        im["wo8"] = sbuf_w(wo_core[:2 * HD, :].astype(f8))
        im["wo16"] = sbuf_w(wo_core.astype(f16))
        in_maps.append(im)
    return in_maps


_CACHE = {}


def kernel(x, freqs_cos, freqs_sin, mask, wq, wk, wv, wo):
    global LAST_EXEC_NS, LAST_RESULTS
    assert tuple(x.shape) == (B, S, D), x.shape

    in_maps = make_inputs(x, freqs_cos, freqs_sin, mask, wq, wk, wv, wo)

    if "nc" not in _CACHE:
        _CACHE["nc"] = build_program()
    nc = _CACHE["nc"]

    kwargs = {}
    if TRACE:
        kwargs = dict(trace=True, trace_cores=[0])
    res = run_bass_kernel_spmd(nc, in_maps, list(range(N_CORES)), **kwargs)
    LAST_EXEC_NS = res.exec_time_ns
    LAST_RESULTS = res

    acc = np.zeros((B, D, S), np.float32)
    for i in range(N_CORES):
        acc += res.results[i]["yT"].astype(np.float32)
    acc *= 1.0 / (SW * SV)
    y = np.ascontiguousarray(np.transpose(acc, (0, 2, 1)))
    return y

